# revision 17
# baseline (speedup 1.0000x reference)
"""Multi-LoRA batched low-rank adapter kernel for 8 trn2 NeuronCores.

Problem: x [16, 2048, 4096] f32, adapter_ids [16] int, A [64, 4096, 64],
B [64, 64, 4096].  out[b] = (x[b] @ B[id_b].T) @ A[id_b].T * (1/64).

Sharding: data-parallel over batch (2 samples/core); per-sample
adapters are gathered on host (adapter_ids are host-visible and tiny).

Precision (HBM traffic 35 MB/core vs 69 MB all-bf16; gate is 2e-2):
 * x ships as fp8 E3M4 (bit-compatible with ml_dtypes.float8_e3m4 on
   device - verified).  mm1 is mixed fp8e3 moving x bf16 stationary.
 * out is stored int8: the scale s_out folded into A on host maps the
   sampled-max |out| to ~108; the DVE/ACT psum drain performs a
   saturating round-to-nearest fp32->int8 (verified on HW); host
   divides by s_out.  End-to-end rel err ~1.4e-2.

PE shape (the budget): with a rank-64 adapter the PE streams 1 column
of 128 values per cycle in BOTH matmuls, so the floor is
(x elems + out elems)/128 = 262k cycles = 109 us/core at 2.4 GHz,
independent of sharding or orientation.  Empirical rules that got
this kernel to ~114-116 us (p10) / ~99 us (paired-median):
 * widest possible moving operands (512; the 1024 fp8 limit is
   rejected by the ISA when psum out is fp32) - narrow matmuls are
   sequencer-dispatch-bound (~100ns/instr; a full-PE-utilization
   x-stationary mm1 with 64-wide moving measured 202 us).
 * narrowest possible stationaries - LDWEIGHTS costs ~1 cycle/column
   and does not overlap (a 128-col pair-packed mm1 stationary
   measured +17 us over this 64-col version).
 * long runs of consecutive same-stationary matmuls and few, big
   drains (finer psum tiles or fine mm1/mm2 interleaving both
   measured slower).
 * mm1: stationary B^T k-tile [128, 64], moving x^T block [128, 512],
   accumulated over 32 k-tiles into a [64, 512] psum -> bx lands
   directly in [rank, seq] layout (no transpose), drained to bf16.
 * mm2: stationary bx chunk [64, 128], moving A^T [64, 512] (K=64,
   no padding), into [128, 1024] psum pairs drained DVE/ACT-
   alternating straight to the int8 staging tile.

Structure: per sample, 4 seq-blocks of 512.  Per block: 4 quarter
x-DMAs (512KB each so mm1 starts early), 32 mm1 matmuls, one DVE
bx-drain, then 4x4 mm2 matmul-pairs + 16 drains, one 2MB SWDGE store
(last block 4x512KB to shorten the tail).  mm2 of block g is emitted
after mm1 of block g+1 so the bx drain hides under PE work.  Adapter
tensors (B^T, A^T per sample) load once and stay resident.
Engine budget/core/iter: PE ~116 us (bound), DMA ~86 us in+out,
DVE ~72 us, ACT ~55 us.
"""

import numpy as np
from contextlib import ExitStack

import concourse.tile as tile
from concourse import bacc, mybir, bass_utils

NCORES = 8
BATCH = 16
B_PER = BATCH // NCORES
SEQ = 2048
DIN = 4096
DOUT = 4096
RANK = 64
SCALE = np.float32(1.0 / 64.0)

f32 = mybir.dt.float32
bf16 = mybir.dt.bfloat16
i8 = mybir.dt.int8
f8e3 = mybir.dt.float8e3

P = 128
KI = DIN // P       # 32 contraction tiles for mm1
SB = 512            # seq block
NBLK = SEQ // SB    # 4
NSB = SB // P       # 4 output row-chunks per block
XQ = 4              # x quarter-DMAs per block
KQ = KI // XQ       # 8 k-tiles per quarter

OUT_TARGET = np.float32(108.0)

_CACHE = {}


def _build_nc(repeat=1):
    nc = bacc.Bacc("TRN2", target_bir_lowering=False, debug=False)
    xq_d = nc.dram_tensor("xq", [B_PER, NBLK, XQ, P, KQ, SB], f8e3,
                          kind="ExternalInput").ap()
    bh_d = nc.dram_tensor("bh", [B_PER, P, KI, RANK], bf16,
                          kind="ExternalInput").ap()
    ah_d = nc.dram_tensor("ah", [B_PER, RANK, DOUT], bf16,
                          kind="ExternalInput").ap()   # A^T * SCALE * s_out
    out = nc.dram_tensor("out", [B_PER, NBLK, P, NSB, DOUT], i8,
                         kind="ExternalOutput").ap()

    with tile.TileContext(nc) as tc, ExitStack() as ctx:
        cstp = ctx.enter_context(tc.tile_pool(name="cst", bufs=1))
        xbp = ctx.enter_context(tc.tile_pool(name="xbp", bufs=2))
        bxsp = ctx.enter_context(tc.tile_pool(name="bxsp", bufs=2))
        stg = ctx.enter_context(tc.tile_pool(name="stg", bufs=2))
        bxps = ctx.enter_context(tc.tile_pool(name="bxps", bufs=2,
                                              space="PSUM"))
        outp = ctx.enter_context(tc.tile_pool(name="outp", bufs=3,
                                              space="PSUM"))

        # Adapter tensors load lazily, just before first use, so the
        # head of the kernel isn't serialized behind 2MB of adapter DMA
        # on the load ring (saves ~8us of a single-shot execution; the
        # steady-state slope is unaffected).
        bhts, ahts = {}, {}

        def get_bht(s):
            if s not in bhts:
                t = cstp.tile([P, KI, RANK], bf16, name=f"bht{s}")
                nc.sync.dma_start(t[:], bh_d[s])
                bhts[s] = t
            return bhts[s]

        def get_aht(s):
            if s not in ahts:
                t = cstp.tile([RANK, DOUT], bf16, name=f"aht{s}")
                nc.sync.dma_start(t[:], ah_d[s])
                ahts[s] = t
            return ahts[s]

        def mm1_block(s, blk):
            bht = get_bht(s)      # before xt: small, and mm1 needs it first
            xt = xbp.tile([P, KI, SB], f8e3, name="xt", tag="xt")
            for q in range(XQ):
                nc.sync.dma_start(xt[:, q * KQ:(q + 1) * KQ, :],
                                  xq_d[s, blk, q])
            bx = bxps.tile([RANK, SB], f32, name="bx", tag="bx")
            for k in range(KI):
                nc.tensor.matmul(bx[:], bht[:, k, :], xt[:, k, :],
                                 start=(k == 0), stop=(k == KI - 1))
            bxh = bxsp.tile([RANK, SB], bf16, name="bxh", tag="bxh")
            nc.vector.tensor_copy(bxh[:], bx[:])
            return bxh

        def mm2_block(s, blk, bxh, last):
            ah = get_aht(s)
            st = stg.tile([P, NSB, DOUT], i8, name="st", tag="st")
            for ns in range(NSB):
                for otp in range(DOUT // 1024):
                    ps = outp.tile([P, 1024], f32, name="ps_o", tag="ps_o")
                    for half in range(2):
                        ov = slice(otp * 1024 + half * 512,
                                   otp * 1024 + (half + 1) * 512)
                        pv = slice(half * 512, (half + 1) * 512)
                        nc.tensor.matmul(ps[:, pv],
                                         bxh[:, ns * P:(ns + 1) * P],
                                         ah[:, ov], start=True, stop=True)
                    dv = slice(otp * 1024, (otp + 1) * 1024)
                    if otp % 2 == 0:
                        nc.vector.tensor_copy(st[:, ns, dv], ps[:])
                    else:
                        nc.scalar.copy(st[:, ns, dv], ps[:])
                nc.gpsimd.dma_start(out[s, blk, :, ns, :], st[:, ns, :])

        blocks = [(s, blk) for _ in range(repeat) for s in range(B_PER)
                  for blk in range(NBLK)]
        prev = None
        for g, (s, blk) in enumerate(blocks):
            bxh = mm1_block(s, blk)
            if prev is not None:
                mm2_block(*prev)
            prev = (s, blk, bxh, g == len(blocks) - 1)
        mm2_block(*prev)
    nc.compile()
    return nc


def _get_nc(repeat=1):
    key = f"nc{repeat}"
    if key not in _CACHE:
        _CACHE[key] = _build_nc(repeat)
    return _CACHE[key]


def _estimate_out_scale(x, ids, A, B):
    """Sampled-rows exact compute -> psum scale for the int8 output."""
    mx = 0.0
    for b in range(BATCH):
        xs = x[b, ::32]                        # [64, DIN]
        o = (xs @ B[ids[b]].T @ A[ids[b]].T) * SCALE
        mx = max(mx, float(np.abs(o).max()))
    return np.float32(OUT_TARGET / mx)


def _prep_in_maps(x, adapter_ids, A, B):
    import ml_dtypes
    x = np.asarray(x, dtype=np.float32)
    ids = np.asarray(adapter_ids).astype(np.int64)
    A = np.asarray(A, dtype=np.float32)
    B = np.asarray(B, dtype=np.float32)

    s_out = _estimate_out_scale(x, ids, A, B)
    As = A * (SCALE * s_out)
    in_maps = []
    for c in range(NCORES):
        sl = slice(c * B_PER, (c + 1) * B_PER)
        cids = ids[sl]
        xT = x[sl].transpose(0, 2, 1)                       # [2, DIN, SEQ]
        # [B_PER, NBLK, XQ, P, KQ, SB]: d = (q*KQ + kq)*P + p, n = blk*SB + m
        xq = xT.reshape(B_PER, XQ, KQ, P, NBLK, SB).transpose(0, 4, 1, 3, 2, 5)
        xq = np.ascontiguousarray(xq).astype(ml_dtypes.float8_e3m4)
        # bh[s, p, k, r] = B^T[k*P + p, r]
        bh = np.stack([
            np.ascontiguousarray(
                B[cids[s]].T.reshape(KI, P, RANK).transpose(1, 0, 2))
            for s in range(B_PER)]).astype(ml_dtypes.bfloat16)
        ah = np.ascontiguousarray(
            As[cids].transpose(0, 2, 1)).astype(ml_dtypes.bfloat16)
        in_maps.append({"xq": xq, "bh": bh, "ah": ah})
    return in_maps, s_out


def kernel(x, adapter_ids, A, B):
    nc = _get_nc()
    in_maps, s_out = _prep_in_maps(x, adapter_ids, A, B)
    res = bass_utils.run_bass_kernel_spmd(
        nc, in_maps, core_ids=list(range(NCORES)))
    out = np.empty((BATCH, SEQ, DOUT), dtype=np.float32)
    inv = np.float32(1.0 / s_out)
    for c in range(NCORES):
        # [B_PER, NBLK, P, NSB, DOUT] -> [B_PER, NBLK, NSB, P, DOUT] -> seq
        o = res.results[c]["out"].astype(np.float32)
        out[c * B_PER:(c + 1) * B_PER] = o.transpose(0, 1, 3, 2, 4).reshape(
            B_PER, SEQ, DOUT) * inv
    return out


# revision 20
# speedup vs baseline: 1.0325x; 1.0325x over previous
"""Multi-LoRA batched low-rank adapter kernel for 8 trn2 NeuronCores.

Problem: x [16, 2048, 4096] f32, adapter_ids [16] int, A [64, 4096, 64],
B [64, 64, 4096].  out[b] = (x[b] @ B[id_b].T) @ A[id_b].T * (1/64).

Sharding: data-parallel over batch (2 samples/core); per-sample
adapters are gathered on host (adapter_ids are host-visible and tiny).

Precision (HBM traffic 35 MB/core vs 69 MB all-bf16; gate is 2e-2):
 * x ships as fp8 E3M4 (bit-compatible with ml_dtypes.float8_e3m4 on
   device - verified).  mm1 is mixed fp8e3 moving x bf16 stationary.
 * out is stored int8: the scale s_out folded into A on host maps the
   sampled-max |out| to ~108; the DVE/ACT psum drain performs a
   saturating round-to-nearest fp32->int8 (verified on HW); host
   divides by s_out.  End-to-end rel err ~1.4e-2.

PE shape (the budget): with a rank-64 adapter the PE streams 1 column
of 128 values per cycle in BOTH matmuls, so the floor is
(x elems + out elems)/128 = 262k cycles = 109 us/core at 2.4 GHz,
independent of sharding or orientation.  Empirical rules that got
this kernel to ~114-116 us (p10) / ~99 us (paired-median):
 * widest possible moving operands (512; the 1024 fp8 limit is
   rejected by the ISA when psum out is fp32) - narrow matmuls are
   sequencer-dispatch-bound (~100ns/instr; a full-PE-utilization
   x-stationary mm1 with 64-wide moving measured 202 us).
 * narrowest possible stationaries - LDWEIGHTS costs ~1 cycle/column
   and does not overlap (a 128-col pair-packed mm1 stationary
   measured +17 us over this 64-col version).
 * long runs of consecutive same-stationary matmuls and few, big
   drains (finer psum tiles or fine mm1/mm2 interleaving both
   measured slower).
 * mm1: stationary B^T k-tile [128, 64], moving x^T block [128, 512],
   accumulated over 32 k-tiles into a [64, 512] psum -> bx lands
   directly in [rank, seq] layout (no transpose), drained to bf16.
 * mm2: stationary bx chunk [64, 128], moving A^T [64, 512] (K=64,
   no padding), into [128, 1024] psum pairs drained DVE/ACT-
   alternating straight to the int8 staging tile.

Structure: per sample, 4 seq-blocks of 512.  Per block: 4 quarter
x-DMAs (512KB each so mm1 starts early), 32 mm1 matmuls, one DVE
bx-drain, then 4x4 mm2 matmul-pairs + 16 drains, four per-ns 512KB
SWDGE stores (spreads store traffic; shortens the final flush).
mm2 of block g is emitted after mm1 of block g+1 so the bx drain
hides under PE work.  Adapter tensors (B^T, A^T per sample) load
lazily at first use - behind the first x block on the load ring -
and stay resident; this plus the per-ns stores cuts a single-shot
execution from 140 to 132 us (sim) while leaving the steady-state
slope at the 110 us floor.
Engine budget/core/iter: PE ~114 us (bound, 99% occupancy in sim),
DMA ~93 us in+out, DVE ~82 us, ACT ~66 us.
"""

import numpy as np
from contextlib import ExitStack

import concourse.tile as tile
from concourse import bacc, mybir, bass_utils

NCORES = 8
BATCH = 16
B_PER = BATCH // NCORES
SEQ = 2048
DIN = 4096
DOUT = 4096
RANK = 64
SCALE = np.float32(1.0 / 64.0)

f32 = mybir.dt.float32
bf16 = mybir.dt.bfloat16
i8 = mybir.dt.int8
f8e3 = mybir.dt.float8e3

P = 128
KI = DIN // P       # 32 contraction tiles for mm1
SB = 512            # seq block
NBLK = SEQ // SB    # 4
NSB = SB // P       # 4 output row-chunks per block
XQ = 4              # x quarter-DMAs per block
KQ = KI // XQ       # 8 k-tiles per quarter

OUT_TARGET = np.float32(108.0)

_CACHE = {}


def _build_nc(repeat=1):
    nc = bacc.Bacc("TRN2", target_bir_lowering=False, debug=False)
    xq_d = nc.dram_tensor("xq", [B_PER, NBLK, XQ, P, KQ, SB], f8e3,
                          kind="ExternalInput").ap()
    bh_d = nc.dram_tensor("bh", [B_PER, P, KI, RANK], bf16,
                          kind="ExternalInput").ap()
    ah_d = nc.dram_tensor("ah", [B_PER, RANK, DOUT], bf16,
                          kind="ExternalInput").ap()   # A^T * SCALE * s_out
    out = nc.dram_tensor("out", [B_PER, NBLK, P, NSB, DOUT], i8,
                         kind="ExternalOutput").ap()

    with tile.TileContext(nc) as tc, ExitStack() as ctx:
        cstp = ctx.enter_context(tc.tile_pool(name="cst", bufs=1))
        xbp = ctx.enter_context(tc.tile_pool(name="xbp", bufs=2))
        bxsp = ctx.enter_context(tc.tile_pool(name="bxsp", bufs=2))
        stg = ctx.enter_context(tc.tile_pool(name="stg", bufs=2))
        bxps = ctx.enter_context(tc.tile_pool(name="bxps", bufs=2,
                                              space="PSUM"))
        outp = ctx.enter_context(tc.tile_pool(name="outp", bufs=3,
                                              space="PSUM"))

        # Adapter tensors load lazily, just before first use, so the
        # head of the kernel isn't serialized behind 2MB of adapter DMA
        # on the load ring (saves ~8us of a single-shot execution; the
        # steady-state slope is unaffected).
        bhts, ahts = {}, {}

        def get_bht(s):
            if s not in bhts:
                t = cstp.tile([P, KI, RANK], bf16, name=f"bht{s}")
                nc.sync.dma_start(t[:], bh_d[s])
                bhts[s] = t
            return bhts[s]

        def get_aht(s):
            if s not in ahts:
                t = cstp.tile([RANK, DOUT], bf16, name=f"aht{s}")
                nc.sync.dma_start(t[:], ah_d[s])
                ahts[s] = t
            return ahts[s]

        def mm1_block(s, blk):
            bht = get_bht(s)      # before xt: small, and mm1 needs it first
            xt = xbp.tile([P, KI, SB], f8e3, name="xt", tag="xt")
            for q in range(XQ):
                nc.sync.dma_start(xt[:, q * KQ:(q + 1) * KQ, :],
                                  xq_d[s, blk, q])
            bx = bxps.tile([RANK, SB], f32, name="bx", tag="bx")
            for k in range(KI):
                nc.tensor.matmul(bx[:], bht[:, k, :], xt[:, k, :],
                                 start=(k == 0), stop=(k == KI - 1))
            bxh = bxsp.tile([RANK, SB], bf16, name="bxh", tag="bxh")
            nc.vector.tensor_copy(bxh[:], bx[:])
            return bxh

        def mm2_block(s, blk, bxh, last):
            ah = get_aht(s)
            store = nc.sync.dma_start if last else nc.gpsimd.dma_start
            st = stg.tile([P, NSB, DOUT], i8, name="st", tag="st")
            for ns in range(NSB):
                for otp in range(DOUT // 1024):
                    ps = outp.tile([P, 1024], f32, name="ps_o", tag="ps_o")
                    for half in range(2):
                        ov = slice(otp * 1024 + half * 512,
                                   otp * 1024 + (half + 1) * 512)
                        pv = slice(half * 512, (half + 1) * 512)
                        nc.tensor.matmul(ps[:, pv],
                                         bxh[:, ns * P:(ns + 1) * P],
                                         ah[:, ov], start=True, stop=True)
                    dv = slice(otp * 1024, (otp + 1) * 1024)
                    if otp % 2 == 0:
                        nc.vector.tensor_copy(st[:, ns, dv], ps[:])
                    else:
                        nc.scalar.copy(st[:, ns, dv], ps[:])
                store(out[s, blk, :, ns, :], st[:, ns, :])

        blocks = [(s, blk) for _ in range(repeat) for s in range(B_PER)
                  for blk in range(NBLK)]
        prev = None
        for g, (s, blk) in enumerate(blocks):
            bxh = mm1_block(s, blk)
            if prev is not None:
                mm2_block(*prev)
            prev = (s, blk, bxh, g >= len(blocks) - 2)
        mm2_block(*prev)
    nc.compile()
    return nc


def _get_nc(repeat=1):
    key = f"nc{repeat}"
    if key not in _CACHE:
        _CACHE[key] = _build_nc(repeat)
    return _CACHE[key]


def _estimate_out_scale(x, ids, A, B):
    """Sampled-rows exact compute -> psum scale for the int8 output."""
    mx = 0.0
    for b in range(BATCH):
        xs = x[b, ::32]                        # [64, DIN]
        o = (xs @ B[ids[b]].T @ A[ids[b]].T) * SCALE
        mx = max(mx, float(np.abs(o).max()))
    return np.float32(OUT_TARGET / mx)


def _prep_in_maps(x, adapter_ids, A, B):
    import ml_dtypes
    x = np.asarray(x, dtype=np.float32)
    ids = np.asarray(adapter_ids).astype(np.int64)
    A = np.asarray(A, dtype=np.float32)
    B = np.asarray(B, dtype=np.float32)

    s_out = _estimate_out_scale(x, ids, A, B)
    As = A * (SCALE * s_out)
    in_maps = []
    for c in range(NCORES):
        sl = slice(c * B_PER, (c + 1) * B_PER)
        cids = ids[sl]
        xT = x[sl].transpose(0, 2, 1)                       # [2, DIN, SEQ]
        # [B_PER, NBLK, XQ, P, KQ, SB]: d = (q*KQ + kq)*P + p, n = blk*SB + m
        xq = xT.reshape(B_PER, XQ, KQ, P, NBLK, SB).transpose(0, 4, 1, 3, 2, 5)
        xq = np.ascontiguousarray(xq).astype(ml_dtypes.float8_e3m4)
        # bh[s, p, k, r] = B^T[k*P + p, r]
        bh = np.stack([
            np.ascontiguousarray(
                B[cids[s]].T.reshape(KI, P, RANK).transpose(1, 0, 2))
            for s in range(B_PER)]).astype(ml_dtypes.bfloat16)
        ah = np.ascontiguousarray(
            As[cids].transpose(0, 2, 1)).astype(ml_dtypes.bfloat16)
        in_maps.append({"xq": xq, "bh": bh, "ah": ah})
    return in_maps, s_out


def kernel(x, adapter_ids, A, B):
    nc = _get_nc()
    in_maps, s_out = _prep_in_maps(x, adapter_ids, A, B)
    res = bass_utils.run_bass_kernel_spmd(
        nc, in_maps, core_ids=list(range(NCORES)))
    out = np.empty((BATCH, SEQ, DOUT), dtype=np.float32)
    inv = np.float32(1.0 / s_out)
    for c in range(NCORES):
        # [B_PER, NBLK, P, NSB, DOUT] -> [B_PER, NBLK, NSB, P, DOUT] -> seq
        o = res.results[c]["out"].astype(np.float32)
        out[c * B_PER:(c + 1) * B_PER] = o.transpose(0, 1, 3, 2, 4).reshape(
            B_PER, SEQ, DOUT) * inv
    return out
